# revision 1
# baseline (speedup 1.0000x reference)
"""Multi-head attention (pre-LN) Bass kernel for 8 Trainium2 NeuronCores.

Problem: y = (softmax(Q K^T / sqrt(dh)) V) Wo + bo with Q/K/V = LN(x) @ Wq/k/v,
x [4, 2048, 1024], 16 heads x 64.

Sharding: 8 cores = 4 batches x 2 head-groups (8 heads each). Each core gets
its batch's x slice and column slices of Wq/Wk/Wv (rows of Wo) and produces a
partial y [2048, 1024]; the host sums the two head-group partials and adds bo.

Per-core pipeline (all matmul operands float32r = full-rate PE, ~1.6e-4 rel):
  1. LN in [token, feature] layout (bn_stats/bn_aggr), PE-transpose to
     xnT [feature, token].
  2. qT/kT = W^T @ xnT (weights stationary); v in natural [token, dh] layout,
     augmented with a ones column per head for softmax sums.
  3. scoresT[j, i] = kT^T-chunk @ qT  (two heads row-packed via tile_position),
     exp on ACT straight from PSUM with the 1/8 scale fused (scores are
     bounded, so no max-subtraction is needed), V-matmul accumulates
     ohT_unnorm[d, i] plus a sums row.
  4. normalize via reciprocal + select-matrix matmul broadcast, then
     y = oT^T-chunks @ Wo.
"""
import os
import sys
from contextlib import ExitStack

import numpy as np

for p in ("/opt/trn_rl_repo/concourse", "/opt/trn_rl_repo"):
    if p not in sys.path:
        sys.path.insert(0, p)

import concourse.bass as bass
import concourse.mybir as mybir
import concourse.tile as tile
from concourse import bacc
from concourse.bass_utils import run_bass_kernel_spmd
from concourse.masks import make_identity

B = 4
S = 2048
DIM = 1024
H = 16
DH = 64
H_LOC = 8              # heads per core
JLOC = H_LOC * DH      # 512: per-core qkv width
EPS = 1e-6
SCALE = DH ** -0.5

F32 = mybir.dt.float32
F32R = mybir.dt.float32r
AF = mybir.ActivationFunctionType
ALU = mybir.AluOpType

NT = S // 128          # 16 token tiles
NTC = 4                # token chunks of 512
NF = DIM // 128        # 8 feature tiles
NMJ = JLOC // 128      # 4 head-pair tiles (2 heads each)

_NC_CACHE = {}


def _build_nc():
    nc = bacc.Bacc("TRN2", target_bir_lowering=False)
    xd = nc.dram_tensor("x", [S, DIM], F32, kind="ExternalInput")
    wqd = nc.dram_tensor("wq", [DIM, JLOC], F32R, kind="ExternalInput")
    wkd = nc.dram_tensor("wk", [DIM, JLOC], F32R, kind="ExternalInput")
    wvd = nc.dram_tensor("wv", [DIM, JLOC], F32R, kind="ExternalInput")
    wod = nc.dram_tensor("wo", [JLOC, DIM], F32R, kind="ExternalInput")
    yd = nc.dram_tensor("y", [S, DIM], F32, kind="ExternalOutput")

    # select matrix: bcast[c, t] = recip[head(c), t] via sel^T @ recip
    sel = np.zeros((H_LOC, JLOC), np.float32)
    for c in range(JLOC):
        mj, cc = divmod(c, 128)
        sel[2 * mj + cc // DH, c] = 1.0
    seld = nc.inline_tensor(sel, "selc")
    onesd = nc.inline_tensor(np.ones((1,), np.float32), "onesc")

    with tile.TileContext(nc) as tc, ExitStack() as ctx:
        pers = ctx.enter_context(tc.tile_pool(name="pers", bufs=1))
        qt_sb = [pers.tile([128, S], F32R, name=f"qt{m}") for m in range(NMJ)]
        kt_sb = [pers.tile([128, S], F32R, name=f"kt{m}") for m in range(NMJ)]
        va_sb = [pers.tile([128, H_LOC, DH + 1], F32R, name=f"va{t}") for t in range(NT)]
        ident = pers.tile([128, 128], F32)
        make_identity(nc, ident)
        eps_sb = pers.tile([128, 1], F32)
        nc.vector.memset(eps_sb, EPS)
        sel_sb = pers.tile([H_LOC, JLOC], F32R)
        nc.gpsimd.dma_start(sel_sb, seld.ap())  # gpsimd: f32 -> f32r cast dma

        # ones column per head in v_aug (drives the softmax sums row)
        for t in range(NT):
            ones_ap = bass.AP(tensor=onesd, offset=0, ap=[[0, 128], [0, H_LOC], [1, 1]])
            nc.gpsimd.dma_start(va_sb[t][:, :, DH:DH + 1], ones_ap)

        # ---------------- phase 1: LN + transpose + QKV ----------------
        with tc.tile_pool(name="p1w", bufs=1) as p1w, \
             tc.tile_pool(name="p1x", bufs=5) as p1x, \
             tc.tile_pool(name="p1t", bufs=1) as p1t, \
             tc.tile_pool(name="p1s", bufs=8) as p1s, \
             tc.tile_pool(name="ps_tr", bufs=3, space="PSUM") as ps_tr, \
             tc.tile_pool(name="ps_qkv", bufs=3, space="PSUM") as ps_qkv:
            wq_sb = p1w.tile([128, NF, JLOC], F32R)
            wk_sb = p1w.tile([128, NF, JLOC], F32R)
            wv_sb = p1w.tile([128, NF, JLOC], F32R)
            nc.sync.dma_start(wq_sb, wqd.ap().rearrange("(ko p) j -> p ko j", p=128))
            nc.sync.dma_start(wk_sb, wkd.ap().rearrange("(ko p) j -> p ko j", p=128))
            nc.sync.dma_start(wv_sb, wvd.ap().rearrange("(ko p) j -> p ko j", p=128))

            for tci in range(NTC):
                tcs = slice(tci * 512, (tci + 1) * 512)
                xts = []
                for tt in range(4):
                    it = tci * 4 + tt
                    xt = p1x.tile([128, DIM], F32, tag="xt")
                    nc.sync.dma_start(xt, xd.ap()[it * 128:(it + 1) * 128, :])
                    st = p1s.tile([128, 2, 6], F32, tag="st")
                    nc.vector.bn_stats(st[:, 0, :], xt[:, 0:512])
                    nc.vector.bn_stats(st[:, 1, :], xt[:, 512:1024])
                    mv = p1s.tile([128, 2], F32, tag="mv")
                    nc.vector.bn_aggr(mv, st)
                    std = p1s.tile([128, 1], F32, tag="std")
                    nc.scalar.activation(std, mv[:, 1:2], AF.Sqrt, bias=eps_sb)
                    rstd = p1s.tile([128, 1], F32, tag="rstd")
                    nc.vector.reciprocal(rstd, std)
                    # x <- (x - mean) * rstd, in place
                    nc.vector.tensor_scalar(
                        out=xt, in0=xt, scalar1=mv[:, 0:1], scalar2=rstd,
                        op0=ALU.subtract, op1=ALU.mult)
                    xts.append(xt)

                # transpose the normalized chunk: xnt[fo] = [feature128, token512]
                xnt = []
                for fo in range(NF):
                    tp = ps_tr.tile([128, 4, 128], F32, tag="tr")
                    for tt in range(4):
                        nc.tensor.transpose(
                            tp[:, tt, :], xts[tt][:, fo * 128:(fo + 1) * 128], ident)
                    xq = p1t.tile([128, 512], F32R, tag=f"xnt{fo}")
                    nc.any.tensor_copy(out=xq, in_=tp)
                    xnt.append(xq)

                # qT / kT for this token chunk
                for wsb, dst in ((wq_sb, qt_sb), (wk_sb, kt_sb)):
                    for m in range(NMJ):
                        pq = ps_qkv.tile([128, 512], F32, tag="pq")
                        for fo in range(NF):
                            nc.tensor.matmul(
                                pq, lhsT=wsb[:, fo, m * 128:(m + 1) * 128],
                                rhs=xnt[fo], start=(fo == 0), stop=(fo == NF - 1))
                        nc.any.tensor_copy(out=dst[m][:, tcs], in_=pq)
                # v in [token, dh] layout, written into the strided v_aug slots
                for tt in range(4):
                    it = tci * 4 + tt
                    pv = ps_qkv.tile([128, 512], F32, tag="pq")
                    for fo in range(NF):
                        nc.tensor.matmul(
                            pv, lhsT=xnt[fo][:, tt * 128:(tt + 1) * 128],
                            rhs=wv_sb[:, fo, :], start=(fo == 0), stop=(fo == NF - 1))
                    nc.any.tensor_copy(
                        out=va_sb[it][:, :, 0:DH],
                        in_=pv.rearrange("p (h d) -> p h d", h=H_LOC))

        # ---------------- phase 2: attention ----------------
        with tc.tile_pool(name="p2", bufs=1) as p2, \
             tc.tile_pool(name="p2e", bufs=3) as p2e, \
             tc.tile_pool(name="p2st", bufs=4) as p2st:
            wo_sb = p2.tile([128, NMJ, DIM], F32R)
            nc.sync.dma_start(wo_sb, wod.ap().rearrange("(co p) m -> p co m", p=128))
            ot_sb = [p2.tile([128, S], F32R, name=f"ot{m}") for m in range(NMJ)]
            sums_sb = p2.tile([H_LOC, S], F32)

            with tc.tile_pool(name="ps_sc", bufs=3, space="PSUM") as ps_sc, \
                 tc.tile_pool(name="ps_o", bufs=1, space="PSUM") as ps_o:
                for m in range(NMJ):
                    hA, hB = 2 * m, 2 * m + 1
                    for ic in range(NTC):
                        ics = slice(ic * 512, (ic + 1) * 512)
                        po = ps_o.tile([128, 2, 512], F32, tag="po")
                        for jt in range(NT):
                            jsl = slice(jt * 128, (jt + 1) * 128)
                            psc = ps_sc.tile([128, 2, 512], F32, tag="sc")
                            nc.tensor.matmul(
                                psc[:, 0, :], lhsT=kt_sb[m][0:64, jsl],
                                rhs=qt_sb[m][0:64, ics], start=True, stop=True,
                                tile_position=(0, 0))
                            nc.tensor.matmul(
                                psc[:, 1, :], lhsT=kt_sb[m][64:128, jsl],
                                rhs=qt_sb[m][64:128, ics], start=True, stop=True,
                                tile_position=(64, 0))
                            ex = p2e.tile([128, 2, 512], F32R, tag="ex")
                            nc.scalar.activation(ex, psc, AF.Exp, scale=SCALE)
                            nc.tensor.matmul(
                                po[0:DH + 1, 0, :], lhsT=va_sb[jt][:, hA, :],
                                rhs=ex[:, 0, :], start=(jt == 0), stop=(jt == NT - 1),
                                skip_group_check=True)
                            nc.tensor.matmul(
                                po[0:DH + 1, 1, :], lhsT=va_sb[jt][:, hB, :],
                                rhs=ex[:, 1, :], start=(jt == 0), stop=(jt == NT - 1),
                                skip_group_check=True)
                        nc.any.tensor_copy(out=ot_sb[m][0:64, ics], in_=po[0:64, 0, :])
                        nc.any.tensor_copy(out=ot_sb[m][64:128, ics], in_=po[0:64, 1, :])
                        for h, u in ((hA, 0), (hB, 1)):
                            stg = p2st.tile([1, 512], F32, tag="st")
                            nc.vector.tensor_copy(stg, po[DH:DH + 1, u, :])
                            nc.sync.dma_start(sums_sb[h:h + 1, ics], stg)

            # ---------------- phase 3: normalize + out-projection ----------------
            recip32 = p2.tile([H_LOC, S], F32)
            nc.vector.reciprocal(recip32, sums_sb)
            recip_sb = p2.tile([H_LOC, S], F32R)
            nc.vector.tensor_copy(recip_sb, recip32)

            with tc.tile_pool(name="p3y", bufs=3) as p3y, \
                 tc.tile_pool(name="ps_bc", bufs=2, space="PSUM") as ps_bc, \
                 tc.tile_pool(name="ps_y", bufs=3, space="PSUM") as ps_y:
                for m in range(NMJ):
                    for tci in range(NTC):
                        tcs = slice(tci * 512, (tci + 1) * 512)
                        pbc = ps_bc.tile([128, 512], F32, tag="bc")
                        nc.tensor.matmul(
                            pbc, lhsT=sel_sb[:, m * 128:(m + 1) * 128],
                            rhs=recip_sb[:, tcs], start=True, stop=True)
                        nc.vector.tensor_mul(ot_sb[m][:, tcs], ot_sb[m][:, tcs], pbc)
                for it in range(NT):
                    for mc in range(2):
                        py = ps_y.tile([128, 512], F32, tag="py")
                        for m in range(NMJ):
                            nc.tensor.matmul(
                                py, lhsT=ot_sb[m][:, it * 128:(it + 1) * 128],
                                rhs=wo_sb[:, m, mc * 512:(mc + 1) * 512],
                                start=(m == 0), stop=(m == NMJ - 1))
                        yt = p3y.tile([128, 512], F32, tag="yt")
                        nc.any.tensor_copy(out=yt, in_=py)
                        nc.sync.dma_start(
                            yd.ap()[it * 128:(it + 1) * 128, mc * 512:(mc + 1) * 512], yt)

    nc.finalize()
    return nc


def _get_nc():
    if "nc" not in _NC_CACHE:
        _NC_CACHE["nc"] = _build_nc()
    return _NC_CACHE["nc"]


def kernel(x, ln_gamma, ln_beta, Wq, Wk, Wv, Wo, bo, _trace=False, _trace_cores=None):
    x = np.asarray(x, np.float32)
    g = np.asarray(ln_gamma, np.float32)
    bb = np.asarray(ln_beta, np.float32)
    Wq = np.asarray(Wq, np.float32)
    Wk = np.asarray(Wk, np.float32)
    Wv = np.asarray(Wv, np.float32)
    Wo = np.asarray(Wo, np.float32)
    bo = np.asarray(bo, np.float32)
    assert np.all(bb == 0.0), "nonzero LN beta not supported"
    # fold gamma into the projection weights (exact for gamma == 1)
    Wqf = g[:, None] * Wq
    Wkf = g[:, None] * Wk
    Wvf = g[:, None] * Wv

    in_maps = []
    for core in range(8):
        b, hg = divmod(core, 2)
        js = slice(hg * JLOC, (hg + 1) * JLOC)
        in_maps.append({
            "x": np.ascontiguousarray(x[b]),
            "wq": np.ascontiguousarray(Wqf[:, js]),
            "wk": np.ascontiguousarray(Wkf[:, js]),
            "wv": np.ascontiguousarray(Wvf[:, js]),
            "wo": np.ascontiguousarray(Wo[js, :]),
        })

    nc = _get_nc()
    kwargs = {}
    if _trace:
        kwargs = dict(trace=True, trace_cores=_trace_cores or [0])
    res = run_bass_kernel_spmd(nc, in_maps, core_ids=list(range(8)), **kwargs)
    kernel.last_results = res
    parts = [r["y"] for r in res.results]
    y = np.stack([parts[2 * b] + parts[2 * b + 1] for b in range(B)], axis=0) + bo
    return y.astype(np.float32)


kernel.last_results = None


# revision 14
# speedup vs baseline: 1.0031x; 1.0031x over previous
"""Multi-head attention (pre-LN) Bass kernel for 8 Trainium2 NeuronCores.

Problem: y = (softmax(Q K^T / sqrt(dh)) V) Wo + bo with Q/K/V = LN(x) @ Wq/k/v,
x [4, 2048, 1024], 16 heads x 64.

Sharding: 8 cores = 4 batches x 2 head-groups (8 heads each). Each core gets
its batch's x slice and column slices of Wq/Wk/Wv (rows of Wo) and produces a
partial y [2048, 1024]; the host sums the two head-group partials and adds bo.

Per-core pipeline (all matmul operands float32r = full-rate PE, ~1.6e-4 rel):
  1. LN in [token, feature] layout (bn_stats/bn_aggr), PE-transpose to
     xnT [feature, token].
  2. qT/kT = W^T @ xnT (weights stationary); v in natural [token, dh] layout,
     augmented with a ones column per head for softmax sums.
  3. scoresT[j, i] = kT^T-chunk @ qT  (two heads row-packed via tile_position),
     exp on ACT straight from PSUM with the 1/8 scale fused (scores are
     bounded, so no max-subtraction is needed), V-matmul accumulates
     ohT_unnorm[d, i] plus a sums row.
  4. normalize via reciprocal + select-matrix matmul broadcast, then
     y = oT^T-chunks @ Wo.
"""
import os
import sys
from contextlib import ExitStack

import numpy as np

for p in ("/opt/trn_rl_repo/concourse", "/opt/trn_rl_repo"):
    if p not in sys.path:
        sys.path.insert(0, p)

import concourse.bass as bass
import concourse.mybir as mybir
import concourse.tile as tile
from concourse import bacc
from concourse.bass_utils import run_bass_kernel_spmd
from concourse.masks import make_identity

B = 4
S = 2048
DIM = 1024
H = 16
DH = 64
H_LOC = 8              # heads per core
JLOC = H_LOC * DH      # 512: per-core qkv width
EPS = 1e-6
SCALE = DH ** -0.5

F32 = mybir.dt.float32
F32R = mybir.dt.float32r
AF = mybir.ActivationFunctionType
ALU = mybir.AluOpType

NT = S // 128          # 16 token tiles
NTC = 4                # token chunks of 512
NF = DIM // 128        # 8 feature tiles
NMJ = JLOC // 128      # 4 head-pair tiles (2 heads each)

_NC_CACHE = {}


def _build_nc():
    nc = bacc.Bacc("TRN2", target_bir_lowering=False)
    xd = nc.dram_tensor("x", [S, DIM], F32, kind="ExternalInput")
    wqd = nc.dram_tensor("wq", [DIM, JLOC], F32R, kind="ExternalInput")
    wkd = nc.dram_tensor("wk", [DIM, JLOC], F32R, kind="ExternalInput")
    wvd = nc.dram_tensor("wv", [DIM, JLOC], F32R, kind="ExternalInput")
    wod = nc.dram_tensor("wo", [JLOC, DIM], F32R, kind="ExternalInput")
    yd = nc.dram_tensor("y", [S, DIM], F32, kind="ExternalOutput")

    # select matrix: bcast[c, t] = recip[c // DH, t] via sel^T @ recip
    sel = np.zeros((2, 128), np.float32)
    sel[0, 0:DH] = 1.0
    sel[1, DH:128] = 1.0
    seld = nc.inline_tensor(sel, "selc")
    onesd = nc.inline_tensor(np.ones((1,), np.float32), "onesc")

    with tile.TileContext(nc) as tc, ExitStack() as ctx:
        pers = ctx.enter_context(tc.tile_pool(name="pers", bufs=1))
        qt_sb = [pers.tile([128, S], F32R, name=f"qt{m}") for m in range(NMJ)]
        kt_sb = [pers.tile([128, S], F32R, name=f"kt{m}") for m in range(NMJ)]
        va_sb = [pers.tile([128, H_LOC, DH + 1], F32R, name=f"va{t}") for t in range(NT)]
        ident = pers.tile([128, 128], F32)
        make_identity(nc, ident)
        eps_sb = pers.tile([128, 1], F32)
        nc.vector.memset(eps_sb, EPS)
        sel_sb = pers.tile([2, 128], F32R)
        nc.gpsimd.dma_start(sel_sb, seld.ap())  # gpsimd: f32 -> f32r cast dma

        # ones column per head in v_aug (drives the softmax sums row)
        for t in range(NT):
            ones_ap = bass.AP(tensor=onesd, offset=0, ap=[[0, 128], [0, H_LOC], [1, 1]])
            nc.gpsimd.dma_start(va_sb[t][:, :, DH:DH + 1], ones_ap)

        # ---------------- phase 1: LN + transpose + QKV ----------------
        with tc.tile_pool(name="p1w", bufs=1) as p1w, \
             tc.tile_pool(name="p1x", bufs=5) as p1x, \
             tc.tile_pool(name="p1t", bufs=1) as p1t, \
             tc.tile_pool(name="p1s", bufs=8) as p1s, \
             tc.tile_pool(name="ps_tr", bufs=3, space="PSUM") as ps_tr, \
             tc.tile_pool(name="ps_qkv", bufs=5, space="PSUM") as ps_qkv:
            wq_sb = p1w.tile([128, NF, JLOC], F32R)
            wk_sb = p1w.tile([128, NF, JLOC], F32R)
            wv_sb = p1w.tile([128, NF, JLOC], F32R)
            nc.sync.dma_start(wq_sb, wqd.ap().rearrange("(ko p) j -> p ko j", p=128))
            nc.sync.dma_start(wk_sb, wkd.ap().rearrange("(ko p) j -> p ko j", p=128))
            nc.sync.dma_start(wv_sb, wvd.ap().rearrange("(ko p) j -> p ko j", p=128))

            for tci in range(NTC):
                tcs = slice(tci * 512, (tci + 1) * 512)
                xts = []
                for tt in range(4):
                    it = tci * 4 + tt
                    xt = p1x.tile([128, DIM], F32, tag="xt")
                    nc.sync.dma_start(xt, xd.ap()[it * 128:(it + 1) * 128, :])
                    st = p1s.tile([128, 2, 6], F32, tag="st")
                    nc.vector.bn_stats(st[:, 0, :], xt[:, 0:512])
                    nc.vector.bn_stats(st[:, 1, :], xt[:, 512:1024])
                    mv = p1s.tile([128, 2], F32, tag="mv")
                    nc.vector.bn_aggr(mv, st)
                    std = p1s.tile([128, 1], F32, tag="std")
                    nc.scalar.activation(std, mv[:, 1:2], AF.Sqrt, bias=eps_sb)
                    rstd = p1s.tile([128, 1], F32, tag="rstd")
                    nc.vector.reciprocal(rstd, std)
                    # x <- (x - mean) * rstd, in place
                    nc.vector.tensor_scalar(
                        out=xt, in0=xt, scalar1=mv[:, 0:1], scalar2=rstd,
                        op0=ALU.subtract, op1=ALU.mult)
                    xts.append(xt)

                # transpose the normalized chunk: xnt[fo] = [feature128, token512]
                xnt = []
                for fo in range(NF):
                    tp = ps_tr.tile([128, 4, 128], F32, tag="tr")
                    for tt in range(4):
                        nc.tensor.transpose(
                            tp[:, tt, :], xts[tt][:, fo * 128:(fo + 1) * 128], ident)
                    xq = p1t.tile([128, 512], F32R, tag=f"xnt{fo}")
                    nc.vector.tensor_copy(xq, tp)
                    xnt.append(xq)

                # qT / kT for this token chunk
                for wsb, dst in ((wq_sb, qt_sb), (wk_sb, kt_sb)):
                    for m in range(NMJ):
                        pq = ps_qkv.tile([128, 512], F32, tag="pq")
                        for fo in range(NF):
                            nc.tensor.matmul(
                                pq, lhsT=wsb[:, fo, m * 128:(m + 1) * 128],
                                rhs=xnt[fo], start=(fo == 0), stop=(fo == NF - 1))
                        nc.vector.tensor_copy(dst[m][:, tcs], pq)
                # v in [token, dh] layout, written into the strided v_aug slots
                for tt in range(4):
                    it = tci * 4 + tt
                    pv = ps_qkv.tile([128, 512], F32, tag="pq")
                    for fo in range(NF):
                        nc.tensor.matmul(
                            pv, lhsT=xnt[fo][:, tt * 128:(tt + 1) * 128],
                            rhs=wv_sb[:, fo, :], start=(fo == 0), stop=(fo == NF - 1))
                    nc.vector.tensor_copy(
                        va_sb[it][:, :, 0:DH],
                        pv.rearrange("p (h d) -> p h d", h=H_LOC))

        # ---------------- phase 2: attention ----------------
        with tc.tile_pool(name="p2", bufs=1) as p2, \
             tc.tile_pool(name="p2e", bufs=3) as p2e, \
             tc.tile_pool(name="p2st", bufs=4) as p2st, \
             tc.tile_pool(name="p2r", bufs=2) as p2r:
            wo_sb = p2.tile([128, NMJ, DIM], F32R)
            nc.sync.dma_start(wo_sb, wod.ap().rearrange("(co p) m -> p co m", p=128))
            ot_sb = [p2.tile([128, S], F32R, name=f"ot{m}") for m in range(NMJ)]
            # head-pair m's sums rows live at partitions {32m, 32m+1} so the
            # per-pair reciprocal slice has a legal 32-multiple base partition
            sums_sb = p2.tile([128, S], F32)

            # scores/exp/V pipeline: 32 bank-slices per (m, ic) (16 j-tiles x
            # {headA, headB}) grouped 3 per PSUM tile so each ACT exp call
            # covers 1536 elems/partition (amortizes the ~352-cycle overhead)
            with tc.tile_pool(name="ps_sc", bufs=2, space="PSUM") as ps_sc, \
                 tc.tile_pool(name="ps_o", bufs=1, space="PSUM") as ps_o:
                SLOTS = 3

                def normalize(m):
                    # recip -> sel-matmul partition broadcast -> elementwise
                    # multiply; emitted one head-pair behind the attention loop
                    # so the sums DMA/recip chain never stalls the PE stream
                    for tci in range(NTC):
                        tcs = slice(tci * 512, (tci + 1) * 512)
                        # NB: reciprocal_approx_* misreads shifted-base inputs
                        # (custom DVE op); exact reciprocal handles them.
                        r32 = p2r.tile([2, 512], F32, tag="r32")
                        nc.vector.reciprocal(r32, sums_sb[32 * m:32 * m + 2, tcs])
                        rr = p2r.tile([2, 512], F32R, tag="rr")
                        nc.vector.tensor_copy(rr, r32)
                        pbc = ps_sc.tile([128, SLOTS, 512], F32, tag="sc")
                        nc.tensor.matmul(
                            pbc[:, 0, :], lhsT=sel_sb, rhs=rr,
                            start=True, stop=True)
                        nc.vector.tensor_mul(
                            ot_sb[m][:, tcs], ot_sb[m][:, tcs], pbc[:, 0, :])

                for m in range(NMJ):
                    hA, hB = 2 * m, 2 * m + 1
                    for ic in range(NTC):
                        ics = slice(ic * 512, (ic + 1) * 512)
                        po = ps_o.tile([128, 2, 512], F32, tag="po")
                        for s0 in range(0, 2 * NT, SLOTS):
                            width = min(SLOTS, 2 * NT - s0)
                            psc = ps_sc.tile([128, SLOTS, 512], F32, tag="sc")
                            for s in range(s0, s0 + width):
                                jt, u = divmod(s, 2)
                                jsl = slice(jt * 128, (jt + 1) * 128)
                                pb = slice(0, 64) if u == 0 else slice(64, 128)
                                nc.tensor.matmul(
                                    psc[:, s - s0, :], lhsT=kt_sb[m][pb, jsl],
                                    rhs=qt_sb[m][pb, ics], start=True, stop=True,
                                    tile_position=(0 if u == 0 else 64, 0))
                            ex = p2e.tile([128, SLOTS, 512], F32R, tag="ex")
                            nc.scalar.activation(
                                ex[:, 0:width, :], psc[:, 0:width, :],
                                AF.Exp, scale=SCALE)
                            for s in range(s0, s0 + width):
                                jt, u = divmod(s, 2)
                                h = hA if u == 0 else hB
                                nc.tensor.matmul(
                                    po[0:DH + 1, u, :], lhsT=va_sb[jt][:, h, :],
                                    rhs=ex[:, s - s0, :],
                                    start=(jt == 0), stop=(jt == NT - 1),
                                    skip_group_check=True)
                        nc.vector.tensor_copy(ot_sb[m][0:64, ics], po[0:64, 0, :])
                        nc.vector.tensor_copy(ot_sb[m][64:128, ics], po[0:64, 1, :])
                        for u in (0, 1):
                            stg = p2st.tile([1, 512], F32, tag="st")
                            nc.vector.tensor_copy(stg, po[DH:DH + 1, u, :])
                            nc.sync.dma_start(
                                sums_sb[32 * m + u:32 * m + u + 1, ics], stg)
                    if m > 0:
                        normalize(m - 1)
                normalize(NMJ - 1)

            # ---------------- phase 3: out-projection ----------------
            with tc.tile_pool(name="p3y", bufs=3) as p3y, \
                 tc.tile_pool(name="ps_y", bufs=3, space="PSUM") as ps_y:
                for it in range(NT):
                    for mc in range(2):
                        py = ps_y.tile([128, 512], F32, tag="py")
                        for m in range(NMJ):
                            nc.tensor.matmul(
                                py, lhsT=ot_sb[m][:, it * 128:(it + 1) * 128],
                                rhs=wo_sb[:, m, mc * 512:(mc + 1) * 512],
                                start=(m == 0), stop=(m == NMJ - 1))
                        yt = p3y.tile([128, 512], F32, tag="yt")
                        nc.vector.tensor_copy(yt, py)
                        nc.sync.dma_start(
                            yd.ap()[it * 128:(it + 1) * 128, mc * 512:(mc + 1) * 512], yt)

    nc.finalize()
    return nc


def _get_nc():
    if "nc" not in _NC_CACHE:
        _NC_CACHE["nc"] = _build_nc()
    return _NC_CACHE["nc"]


def kernel(x, ln_gamma, ln_beta, Wq, Wk, Wv, Wo, bo, _trace=False, _trace_cores=None):
    x = np.asarray(x, np.float32)
    g = np.asarray(ln_gamma, np.float32)
    bb = np.asarray(ln_beta, np.float32)
    Wq = np.asarray(Wq, np.float32)
    Wk = np.asarray(Wk, np.float32)
    Wv = np.asarray(Wv, np.float32)
    Wo = np.asarray(Wo, np.float32)
    bo = np.asarray(bo, np.float32)
    assert np.all(bb == 0.0), "nonzero LN beta not supported"
    # fold gamma into the projection weights (exact for gamma == 1)
    Wqf = g[:, None] * Wq
    Wkf = g[:, None] * Wk
    Wvf = g[:, None] * Wv

    in_maps = []
    for core in range(8):
        b, hg = divmod(core, 2)
        js = slice(hg * JLOC, (hg + 1) * JLOC)
        in_maps.append({
            "x": np.ascontiguousarray(x[b]),
            "wq": np.ascontiguousarray(Wqf[:, js]),
            "wk": np.ascontiguousarray(Wkf[:, js]),
            "wv": np.ascontiguousarray(Wvf[:, js]),
            "wo": np.ascontiguousarray(Wo[js, :]),
        })

    nc = _get_nc()
    kwargs = {}
    if _trace:
        kwargs = dict(trace=True, trace_cores=_trace_cores or [0])
    res = run_bass_kernel_spmd(nc, in_maps, core_ids=list(range(8)), **kwargs)
    kernel.last_results = res
    parts = [r["y"] for r in res.results]
    y = np.stack([parts[2 * b] + parts[2 * b + 1] for b in range(B)], axis=0) + bo
    return y.astype(np.float32)


kernel.last_results = None
